# revision 21
# baseline (speedup 1.0000x reference)
"""Causal self-attention (GQA + RoPE) Trainium2 kernel, bf16 tensor-core path.

Full-input contract: kernel(**inputs) takes the unsharded tensors and returns
the full [B, T, C] output. Internally shards over 8 NeuronCores as
(batch b in {0,1}) x (kv-head group g in {0..3}); each core computes the
attention output of its 4 query heads (one kv head) for its batch and the
partial out-projection against its 512 rows of Wo. The host sums the 4 group
partials per batch.

v2 design (vs the fp32r baseline):
  - all matmul operands bf16 (fp32 PSUM accumulation). Host converts inputs.
  - phase A (projections) and phase B (attention q-blocks) are interleaved
    chunk-wise in emission order so the PE never idles long enough to drop
    out of its ramped p-state: A0 A1 B0a A2 B0o B1a A3 B1o B2a B3a B2o B3o.
  - PSUM budget kept at 8 banks at every point: A accumulators 3 (two passes
    per t-chunk: {q0,q1,k} then {q2,q3,v}), scores/outproj 3, denom 1, attnV 1.
  - V is produced directly in [t, d] layout (x-chunk stationary, Wv moving),
    no PE transpose pass.
  - causal masking via a 0/1 lower-triangle multiply on the exp output (bf16,
    SBUF) instead of a -1e30 add on the fp32 PSUM scores; scores, exp, the
    denominator matmul and attn@V are all restricted to the causally valid
    column range [f0:TC] of each k-tile (accumulation regions only shrink
    after the full-width kt=0 tile, so partial-range PSUM accumulate is safe).
"""

import sys

for _p in ("/opt/trn_rl_repo", "/root/.axon_site/_ro/trn_rl_repo"):
    if _p not in sys.path:
        sys.path.append(_p)

import numpy as np
import ml_dtypes
from contextlib import ExitStack

import concourse.bass as bass
import concourse.bacc as bacc
import concourse.tile as tile
import concourse.mybir as mybir
from concourse.bass_utils import run_bass_kernel_spmd

F32 = mybir.dt.float32
BF16 = mybir.dt.bfloat16
NPBF16 = ml_dtypes.bfloat16

B, T, C = 2, 2048, 2048
N_HEADS, N_KV_HEADS, HD = 16, 4, 128
G = N_HEADS // N_KV_HEADS  # heads per group = 4
GW = G * HD  # 512, per-group Q width / Wo row count
N_CORES = 8
TC = 512  # q-block width
NTC = T // TC  # 4
NKT = T // HD  # 16 k-tiles of 128
NCC = C // 128  # 16 contraction chunks

_prog_cache = {}


def _build_program():
    nc = bacc.Bacc(
        "TRN2",
        target_bir_lowering=False,
        debug=False,
        enable_asserts=False,
        num_devices=N_CORES,
    )

    xT = nc.dram_tensor("xT", [C, T], BF16, kind="ExternalInput").ap()
    wq = nc.dram_tensor("wq", [128, NCC * GW], BF16, kind="ExternalInput").ap()
    wk = nc.dram_tensor("wk", [128, NCC * HD], BF16, kind="ExternalInput").ap()
    wv = nc.dram_tensor("wv", [128, NCC * HD], BF16, kind="ExternalInput").ap()
    wo = nc.dram_tensor("wo", [128, G * C], BF16, kind="ExternalInput").ap()
    cos = nc.dram_tensor("cos", [HD, T], BF16, kind="ExternalInput").ap()
    sin = nc.dram_tensor("sin", [HD, T], BF16, kind="ExternalInput").ap()
    tri = nc.dram_tensor("tri", [128, 128], BF16, kind="ExternalInput").ap()
    ones = nc.dram_tensor("ones", [128, 128], BF16, kind="ExternalInput").ap()
    y = nc.dram_tensor("y", [T, C], BF16, kind="ExternalOutput").ap()

    with tile.TileContext(nc) as tc, ExitStack() as ctx:
        cpool = ctx.enter_context(tc.tile_pool(name="const", bufs=1))
        big = ctx.enter_context(tc.tile_pool(name="big", bufs=1))
        xin = ctx.enter_context(tc.tile_pool(name="xin", bufs=NCC))
        rp = ctx.enter_context(tc.tile_pool(name="rp", bufs=4))
        ptp = ctx.enter_context(tc.tile_pool(name="pt", bufs=8))
        nrm = ctx.enter_context(tc.tile_pool(name="nrm", bufs=2))
        otq = ctx.enter_context(tc.tile_pool(name="otq", bufs=2))
        ysb = ctx.enter_context(tc.tile_pool(name="ysb", bufs=4))

        aps = ctx.enter_context(tc.tile_pool(name="aps", bufs=3, space="PSUM"))
        stp = ctx.enter_context(tc.tile_pool(name="stp", bufs=3, space="PSUM"))
        sbp = ctx.enter_context(tc.tile_pool(name="sbp", bufs=1, space="PSUM"))
        otp = ctx.enter_context(tc.tile_pool(name="otp", bufs=1, space="PSUM"))

        # ------------- constants / weights -------------
        # one tile per DMA so dependency tracking is unambiguous
        wq_q = [cpool.tile([128, 4 * GW], BF16, name=f"wqq{q}") for q in range(4)]
        wk_sb = cpool.tile([128, NCC * HD], BF16)
        wv_sb = cpool.tile([128, NCC * HD], BF16)
        wo_h = [cpool.tile([128, C], BF16, name=f"woh{h}") for h in range(G)]
        cos_sb = cpool.tile([HD, T], BF16)
        sin_sb = cpool.tile([HD, T], BF16)
        tri_sb = cpool.tile([128, 128], BF16)
        ones_sb = cpool.tile([128, 128], BF16)

        # big activations: QT [d, h*T + t], KT [d, t], V [t-part, kt*HD + d]
        qt_sb = big.tile([128, G * T], BF16)
        kt_sb = big.tile([128, T], BF16)
        v_sb = big.tile([128, NKT * HD], BF16)

        # x as 32 [128, 2*TC] slabs covering t-block pairs {0,1} and {2,3}:
        # same bytes as 512-wide slabs but half the DMA issues and completion
        # waits, so A0's supply outruns the PE while A1 rides along for free
        x_sb = [
            [
                xin.tile([128, 2 * TC], BF16, tag="x", name=f"x{pr}_{ci}")
                for ci in range(NCC)
            ]
            for pr in range(2)
        ]

        # -- prefetch DMAs, consumption-ordered. Sync carries weights plus
        # the odd x slabs of the live t-block; gpsimd carries the even x
        # slabs. Later t-blocks stream behind the first.
        def xdma(pr, ci):
            eng = nc.sync if ci % 2 else nc.gpsimd
            eng.dma_start(
                x_sb[pr][ci][:],
                xT[ci * 128 : (ci + 1) * 128, pr * 2 * TC : (pr + 1) * 2 * TC],
            )

        nc.sync.dma_start(wk_sb[:, 0 : 8 * HD], wk[:, 0 : 8 * HD])
        nc.sync.dma_start(wq_q[0][:], wq[:, 0 : 4 * GW])
        xdma(0, 0)
        nc.sync.dma_start(wk_sb[:, 8 * HD :], wk[:, 8 * HD :])
        for ci in [2, 4]:
            xdma(0, ci)
        nc.sync.dma_start(wq_q[1][:], wq[:, 4 * GW : 8 * GW])
        xdma(0, 1)
        xdma(0, 3)
        for ci in [6, 8]:
            xdma(0, ci)
        xdma(0, 5)
        nc.sync.dma_start(wv_sb[:], wv[:])
        xdma(0, 7)
        nc.sync.dma_start(wq_q[2][:], wq[:, 8 * GW : 12 * GW])
        for ci in [10, 12, 14]:
            xdma(0, ci)
        xdma(0, 9)
        xdma(0, 11)
        nc.sync.dma_start(wq_q[3][:], wq[:, 12 * GW : 16 * GW])
        xdma(0, 13)
        xdma(0, 15)
        nc.sync.dma_start(cos_sb[:], cos[:])
        nc.sync.dma_start(sin_sb[:], sin[:])
        nc.sync.dma_start(tri_sb[:], tri[:])
        nc.sync.dma_start(ones_sb[:], ones[:])
        for h in range(G):
            nc.sync.dma_start(wo_h[h][:], wo[:, h * C : (h + 1) * C])

        def wq_st(ci, j):
            q, cl = divmod(ci, 4)
            return wq_q[q][:, cl * GW + j * HD : cl * GW + (j + 1) * HD]

        def a_chunk(tci):
            """Projections for t-chunk tci: QT heads, KT, and V in [t,d]."""
            if tci == 2:
                for ci in range(NCC):
                    xdma(1, ci)
            xt = x_sb[tci // 2]
            toff = (tci % 2) * TC
            ts = slice(tci * TC, (tci + 1) * TC)
            # pass 1: q0, q1, k
            q01 = [
                aps.tile([128, TC], F32, tag="aps", name=f"qtps{tci}_{j}")
                for j in range(2)
            ]
            kt_ps = aps.tile([128, TC], F32, tag="aps", name=f"ktps{tci}")
            for ci in range(NCC):
                st, sp = (ci == 0), (ci == NCC - 1)
                for j in range(2):
                    nc.tensor.matmul(
                        q01[j][:], wq_st(ci, j), xt[ci][:, toff : toff + TC], start=st, stop=sp
                    )
                nc.tensor.matmul(
                    kt_ps[:],
                    wk_sb[:, ci * HD : (ci + 1) * HD],
                    xt[ci][:, toff : toff + TC],
                    start=st,
                    stop=sp,
                )
            # pass 2: q2, q3, v(direct [t,d] via x-stationary)
            q23 = [
                aps.tile([128, TC], F32, tag="aps", name=f"qtps{tci}_{j + 2}")
                for j in range(2)
            ]
            v_ps = aps.tile([128, TC], F32, tag="aps", name=f"vtps{tci}")
            for ci in range(NCC):
                st, sp = (ci == 0), (ci == NCC - 1)
                for j in range(2):
                    nc.tensor.matmul(
                        q23[j][:], wq_st(ci, j + 2), xt[ci][:, toff : toff + TC], start=st, stop=sp
                    )
                # one psum accumulation group for the whole bank: start only on
                # the very first sub-write (marks the full 2KB zero region),
                # stop only on the very last
                for s in range(TC // 128):
                    nc.tensor.matmul(
                        v_ps[:, s * HD : (s + 1) * HD],
                        xt[ci][:, toff + s * 128 : toff + (s + 1) * 128],
                        wv_sb[:, ci * HD : (ci + 1) * HD],
                        start=(st and s == 0),
                        stop=(sp and s == TC // 128 - 1),
                        skip_group_check=True,
                    )

            # rope on Q heads: out = q*cos + swap(q)*sin_signed
            qt_ps = q01 + q23
            for j in range(G):
                q_raw = rp.tile([128, TC], BF16, tag="qraw", name=f"qraw{tci}_{j}")
                nc.scalar.copy(q_raw[:], qt_ps[j][:])
                t1 = rp.tile([128, TC], BF16, tag="t1", name=f"t1_{tci}_{j}")
                nc.vector.tensor_mul(t1[:], q_raw[:], cos_sb[:, ts])
                qsw = rp.tile([128, TC], BF16, tag="qsw", name=f"qsw{tci}_{j}")
                nc.gpsimd.dma_start(qsw[0:64, :], q_raw[64:128, :])
                nc.gpsimd.dma_start(qsw[64:128, :], q_raw[0:64, :])
                t2 = rp.tile([128, TC], BF16, tag="t2", name=f"t2_{tci}_{j}")
                nc.vector.tensor_mul(t2[:], qsw[:], sin_sb[:, ts])
                nc.vector.tensor_add(
                    qt_sb[:, j * T + tci * TC : j * T + (tci + 1) * TC], t1[:], t2[:]
                )
            # rope on K
            k_raw = rp.tile([128, TC], BF16, tag="qraw", name=f"kraw{tci}")
            nc.scalar.copy(k_raw[:], kt_ps[:])
            t1k = rp.tile([128, TC], BF16, tag="t1", name=f"t1k{tci}")
            nc.vector.tensor_mul(t1k[:], k_raw[:], cos_sb[:, ts])
            ksw = rp.tile([128, TC], BF16, tag="qsw", name=f"ksw{tci}")
            nc.gpsimd.dma_start(ksw[0:64, :], k_raw[64:128, :])
            nc.gpsimd.dma_start(ksw[64:128, :], k_raw[0:64, :])
            t2k = rp.tile([128, TC], BF16, tag="t2", name=f"t2k{tci}")
            nc.vector.tensor_mul(t2k[:], ksw[:], sin_sb[:, ts])
            nc.vector.tensor_add(kt_sb[:, ts], t1k[:], t2k[:])
            # V psum -> sbuf (already [t, d])
            nc.scalar.copy(v_sb[:, tci * 4 * HD : (tci + 1) * 4 * HD], v_ps[:])

        def b_attn(qb):
            """Attention for q-block qb -> normalized ot_qb [d, h*TC + q]."""
            nkt = (qb + 1) * (TC // 128)
            ot_qb = otq.tile([128, G * TC], BF16, tag="ot", name=f"ot{qb}")
            for h in range(G):
                sb_ps = sbp.tile([128, TC], F32, tag="sb", name=f"sb{qb}_{h}")
                ot_ps = otp.tile([128, TC], F32, tag="otp", name=f"otp{qb}_{h}")
                diag = [kt for kt in range(max(1, 4 * qb), nkt)]
                offd = [kt for kt in range(1, 4 * qb)]
                order = [0] + diag + offd
                for oi, kt in enumerate(order):
                    dj = kt - 4 * qb
                    f0 = max(dj, 0) * 128  # first causally-valid column
                    st, sp = (oi == 0), (oi == nkt - 1)
                    s_t = stp.tile([128, TC], F32, tag="st", name=f"st{qb}_{kt}_{h}")
                    nc.tensor.matmul(
                        s_t[:, f0:TC],
                        kt_sb[:, kt * 128 : (kt + 1) * 128],
                        qt_sb[:, h * T + qb * TC + f0 : h * T + (qb + 1) * TC],
                        start=True,
                        stop=True,
                    )
                    pt = ptp.tile([128, TC], BF16, tag="pt", name=f"pt{qb}_{kt}_{h}")
                    nc.scalar.activation(
                        pt[:, f0:TC],
                        s_t[:, f0:TC],
                        mybir.ActivationFunctionType.Exp,
                    )
                    if dj >= 0:
                        nc.vector.tensor_mul(
                            pt[:, f0 : f0 + 128], pt[:, f0 : f0 + 128], tri_sb[:]
                        )
                    nc.tensor.matmul(
                        sb_ps[:, f0:TC], ones_sb[:], pt[:, f0:TC], start=st, stop=sp
                    )
                    nc.tensor.matmul(
                        ot_ps[:, f0:TC],
                        v_sb[:, kt * HD : (kt + 1) * HD],
                        pt[:, f0:TC],
                        start=st,
                        stop=sp,
                    )
                r_f = nrm.tile([128, TC], F32, tag="rf", name=f"rf{qb}_{h}")
                nc.vector.reciprocal_approx_fast(r_f[:], sb_ps[:])
                nc.vector.tensor_mul(
                    ot_qb[:, h * TC : (h + 1) * TC], ot_ps[:], r_f[:]
                )
            return ot_qb

        def b_outproj(qb, ot_qb):
            for tl in range(TC // 128):
                tsub = qb * (TC // 128) + tl
                for cc in range(C // TC):
                    y_ps = stp.tile([128, TC], F32, tag="st", name=f"yps{tsub}_{cc}")
                    for h in range(G):
                        nc.tensor.matmul(
                            y_ps[:],
                            ot_qb[:, h * TC + tl * 128 : h * TC + (tl + 1) * 128],
                            wo_h[h][:, cc * TC : (cc + 1) * TC],
                            start=(h == 0),
                            stop=(h == G - 1),
                        )
                    y_t = ysb.tile([128, TC], BF16, tag="ysb", name=f"ysb{tsub}_{cc}")
                    # alternate copy engine and DMA queue so neither the DVE
                    # nor a single DMA ring paces the out-projection stream
                    if cc % 2:
                        nc.scalar.copy(y_t[:], y_ps[:])
                    else:
                        nc.vector.tensor_copy(y_t[:], y_ps[:])
                    deng = nc.gpsimd if cc % 2 else nc.sync
                    deng.dma_start(
                        y[tsub * 128 : (tsub + 1) * 128, cc * TC : (cc + 1) * TC],
                        y_t[:],
                    )

        # ---- interleaved schedule: PE stays dense, deps always one block ahead
        a_chunk(0)
        a_chunk(1)
        ot0 = b_attn(0)
        a_chunk(2)
        b_outproj(0, ot0)
        ot1 = b_attn(1)
        a_chunk(3)
        b_outproj(1, ot1)
        ot2 = b_attn(2)
        ot3 = b_attn(3)
        b_outproj(2, ot2)
        b_outproj(3, ot3)

    nc.compile()
    return nc


def _rope_tables():
    theta = 1.0 / (10000.0 ** (np.arange(0, HD, 2, dtype=np.float32) / HD))
    freqs = np.arange(T, dtype=np.float32)[:, None] * theta[None, :]  # [T, 64]
    cos = np.concatenate([np.cos(freqs), np.cos(freqs)], axis=-1)  # [T, 128]
    sin = np.concatenate([np.sin(freqs), np.sin(freqs)], axis=-1)
    cosT = np.ascontiguousarray(cos.T).astype(np.float32)  # [128, T]
    sinT = np.ascontiguousarray(sin.T).astype(np.float32)
    sign = np.where(np.arange(HD) < HD // 2, np.float32(-1.0), np.float32(1.0))[:, None]
    sinT_signed = (sinT * sign).astype(np.float32)
    return cosT.astype(NPBF16), sinT_signed.astype(NPBF16)


def make_in_maps(x, Wq, Wk, Wv, Wo):
    x = np.asarray(x, dtype=np.float32)
    Wq = np.asarray(Wq, dtype=np.float32)
    Wk = np.asarray(Wk, dtype=np.float32)
    Wv = np.asarray(Wv, dtype=np.float32)
    Wo = np.asarray(Wo, dtype=np.float32)

    cosT, sinT = _rope_tables()
    qscale = np.float32(1.0 / np.sqrt(HD))
    p = np.arange(128)[:, None]
    f = np.arange(128)[None, :]
    tri = (p <= f).astype(NPBF16)
    ones = np.ones((128, 128), dtype=NPBF16)

    xb = [np.ascontiguousarray(x[b].T).astype(NPBF16) for b in range(B)]

    def chunk_major(w):
        # [n*128, d] -> [128, n*d] with chunk ci's rows side by side
        n = w.shape[0] // 128
        return np.ascontiguousarray(
            w.reshape(n, 128, w.shape[1]).transpose(1, 0, 2).reshape(128, -1)
        )

    wqb = (Wq * qscale).astype(NPBF16)
    wkb = Wk.astype(NPBF16)
    wvb = Wv.astype(NPBF16)
    wob = Wo.astype(NPBF16)

    in_maps = []
    for c in range(N_CORES):
        b, g = divmod(c, N_KV_HEADS)
        in_maps.append(
            {
                "xT": xb[b],
                "wq": chunk_major(wqb[:, g * GW : (g + 1) * GW]),
                "wk": chunk_major(wkb[:, g * HD : (g + 1) * HD]),
                "wv": chunk_major(wvb[:, g * HD : (g + 1) * HD]),
                "wo": chunk_major(wob[g * GW : (g + 1) * GW, :]),
                "cos": cosT,
                "sin": sinT,
                "tri": tri,
                "ones": ones,
            }
        )
    return in_maps


def kernel(x, Wq, Wk, Wv, Wo):
    if "nc" not in _prog_cache:
        _prog_cache["nc"] = _build_program()
    nc = _prog_cache["nc"]

    in_maps = make_in_maps(x, Wq, Wk, Wv, Wo)
    res = run_bass_kernel_spmd(nc, in_maps, list(range(N_CORES)))
    _prog_cache["last_results"] = res

    out = np.zeros((B, T, C), dtype=np.float32)
    for c in range(N_CORES):
        b = c // N_KV_HEADS
        out[b] += res.results[c]["y"].astype(np.float32)
    return out
